# revision 1
# baseline (speedup 1.0000x reference)
"""Grouped-index Conv1D (moe_routing) on 8 TRN2 NeuronCores.

Math:  out[b,d,t] = sum_c sum_k x[b,c,t+k] * W[gi[b,c],d,k] + count0[b]*bias[d]

Device algorithm (per core, 2 batches, data-parallel over batch):
  1. one-hot M[c,g] = (gi[b,c]==g) built on-chip (iota + tensor_scalar is_equal)
  2. S[g,t] = sum_c M[c,g]*x[c,t]          (PE: one-hot matmul, contraction=256)
  3. out[d,t] = sum_k Wk[g,d]^T S[g,t+k]   (PE: 7 shifted matmuls accumulated
                                            in PSUM, contraction=16)
  4. bias: counts via ones-matmul, count0*bias broadcast via 1-row matmul,
     fused add on DVE during PSUM->SBUF evacuation.
"""

import sys
import numpy as np

sys.path.insert(0, "/opt/trn_rl_repo")

BS, CH, T = 16, 256, 2048
G, D, K = 16, 64, 7
T_OUT = T - K + 1  # 2042
N_CORES = 8
BPC = BS // N_CORES  # batches per core = 2

# matmul input dtype for the two heavy stages: "f32" (exact, 4 cyc/row) or
# "f32r" (fast fp32 PE mode, 1 cyc/row at free>=256)
MM_DTYPE = "f32r"

_COMPILED = {}


def _build(mm_dtype: str):
    from concourse import bacc, tile
    import concourse.mybir as mybir

    f32 = mybir.dt.float32
    f32r = mybir.dt.float32r
    eq = mybir.AluOpType.is_equal
    add = mybir.AluOpType.add

    def mm_ap(ap):
        return ap.bitcast(f32r) if mm_dtype == "f32r" else ap

    nc = bacc.Bacc("TRN2", target_bir_lowering=False, debug=False,
                   num_devices=N_CORES)
    x_ext = nc.dram_tensor("x", [BPC, CH, T], f32, kind="ExternalInput").ap()
    gi_ext = nc.dram_tensor("gi", [BPC, 2, 128, 1], f32, kind="ExternalInput").ap()
    wt_ext = nc.dram_tensor("wt", [G, K * D], f32, kind="ExternalInput").ap()
    b_ext = nc.dram_tensor("bias", [1, D], f32, kind="ExternalInput").ap()
    out_ext = nc.dram_tensor("out", [BPC, D, T_OUT], f32, kind="ExternalOutput").ap()

    NCHUNK = T // 512  # 4

    with tile.TileContext(nc) as tc:
        with (
            tc.tile_pool(name="const", bufs=1) as cpool,
            tc.tile_pool(name="work", bufs=2) as wpool,
            tc.tile_pool(name="ps_pool", bufs=2, space="PSUM") as ppool,
            tc.tile_pool(name="psmall", bufs=1, space="PSUM") as spool,
            tc.tile_pool(name="po_pool", bufs=4, space="PSUM") as opool,
        ):
            wt_sb = cpool.tile([G, K * D], f32, name="wt_sb")
            nc.sync.dma_start(wt_sb[:], wt_ext[:])
            brow = cpool.tile([1, D], f32, name="brow")
            nc.sync.dma_start(brow[:], b_ext[:])
            iota_f = cpool.tile([128, G], f32, name="iota_f")
            nc.gpsimd.iota(iota_f[:], pattern=[[1, G]], base=0,
                           channel_multiplier=0,
                           allow_small_or_imprecise_dtypes=True)
            ones_col = cpool.tile([128, 1], f32, name="ones_col")
            nc.vector.memset(ones_col[:], 1.0)

            for b in range(BPC):
                # --- one-hot + counts + per-batch bias vector ---
                ms = []
                for h in range(2):
                    gi_t = wpool.tile([128, 1], f32, name=f"gi{b}{h}",
                                      tag="gi", bufs=4)
                    nc.sync.dma_start(gi_t[:], gi_ext[b, h])
                    m_t = wpool.tile([128, G], f32, name=f"m{b}{h}",
                                     tag="m", bufs=4)
                    nc.vector.tensor_scalar(out=m_t[:], in0=iota_f[:],
                                            scalar1=gi_t[:, 0:1], scalar2=None,
                                            op0=eq)
                    ms.append(m_t)

                pcnt = spool.tile([G, 1], f32, name=f"pcnt{b}", tag="pcnt")
                nc.tensor.matmul(pcnt[:], ms[0][:], ones_col[:],
                                 start=True, stop=False)
                nc.tensor.matmul(pcnt[:], ms[1][:], ones_col[:],
                                 start=False, stop=True)
                cnt_sb = wpool.tile([G, 1], f32, name=f"cnt{b}", tag="cnt")
                nc.vector.tensor_copy(cnt_sb[:], pcnt[:])
                pbc = spool.tile([D, 1], f32, name=f"pbc{b}", tag="pbc")
                nc.tensor.matmul(pbc[:], brow[:], cnt_sb[0:1, 0:1],
                                 start=True, stop=True)
                bcnt = wpool.tile([D, 1], f32, name=f"bcnt{b}", tag="bcnt")
                nc.vector.tensor_copy(bcnt[:], pbc[:])

                # --- x tiles: 2 channel-halves x 2 column-pieces of 1024 ---
                xp = [[None, None], [None, None]]
                for h in range(2):
                    for p in range(2):
                        t_ = wpool.tile([128, 1024], f32, name=f"xp{b}{h}{p}",
                                        tag="xp", bufs=5)
                        nc.sync.dma_start(
                            t_[:],
                            x_ext[b, 128 * h:128 * (h + 1),
                                  1024 * p:1024 * (p + 1)])
                        xp[h][p] = t_

                # --- S = M^T @ X ---
                s_sb = wpool.tile([G, T], f32, name=f"s{b}", tag="s")
                for c in range(NCHUNK):
                    ps = ppool.tile([G, 512], f32, name=f"ps{b}{c}", tag="ps")
                    off = 512 * c
                    p, o = off // 1024, off % 1024
                    nc.tensor.matmul(ps[:], mm_ap(ms[0][:]),
                                     mm_ap(xp[0][p][:, o:o + 512]),
                                     start=True, stop=False)
                    nc.tensor.matmul(ps[:], mm_ap(ms[1][:]),
                                     mm_ap(xp[1][p][:, o:o + 512]),
                                     start=False, stop=True)
                    nc.vector.tensor_copy(s_sb[:, off:off + 512], ps[:])

                # --- conv: k-accumulated matmuls, then bias-add + store ---
                for c in range(NCHUNK):
                    c0 = 512 * c
                    L = min(512, T_OUT - c0)
                    po = opool.tile([D, 512], f32, name=f"po{b}{c}", tag="po")
                    for k in range(K):
                        nc.tensor.matmul(po[:, :L],
                                         mm_ap(wt_sb[:, D * k:D * (k + 1)]),
                                         mm_ap(s_sb[:, c0 + k:c0 + k + L]),
                                         start=(k == 0), stop=(k == K - 1))
                    osb = wpool.tile([D, 512], f32, name=f"osb{b}{c}",
                                     tag="osb", bufs=3)
                    nc.vector.tensor_scalar(out=osb[:, :L], in0=po[:, :L],
                                            scalar1=bcnt[:, 0:1], scalar2=None,
                                            op0=add)
                    nc.sync.dma_start(out_ext[b, :, c0:c0 + L], osb[:, :L])

    nc.compile()
    return nc


def _get_nc(mm_dtype: str):
    if mm_dtype not in _COMPILED:
        _COMPILED[mm_dtype] = _build(mm_dtype)
    return _COMPILED[mm_dtype]


def _run(x, group_idxs, W, bias, mm_dtype=None, trace=False, tmpdir=None):
    from concourse.bass_utils import run_bass_kernel_spmd

    x = np.ascontiguousarray(np.asarray(x, dtype=np.float32))
    gi = np.asarray(group_idxs).astype(np.float32).reshape(BS, 2, 128, 1)
    W = np.asarray(W, dtype=np.float32)
    bias = np.asarray(bias, dtype=np.float32)
    # wt[g, k*64+d] = W[g,d,k]
    wt = np.ascontiguousarray(W.transpose(0, 2, 1).reshape(G, K * D))
    brow = np.ascontiguousarray(bias.reshape(1, D))

    nc = _get_nc(mm_dtype or MM_DTYPE)
    in_maps = []
    for i in range(N_CORES):
        sl = slice(i * BPC, (i + 1) * BPC)
        in_maps.append({
            "x": np.ascontiguousarray(x[sl]),
            "gi": np.ascontiguousarray(gi[sl]),
            "wt": wt,
            "bias": brow,
        })
    res = run_bass_kernel_spmd(nc, in_maps, core_ids=list(range(N_CORES)),
                               trace=trace, tmpdir=tmpdir)
    out = np.concatenate([r["out"] for r in res.results], axis=0)
    assert out.shape == (BS, D, T_OUT)
    return out.astype(np.float32), res


def kernel(x, group_idxs, W, bias):
    out, _ = _run(x, group_idxs, W, bias)
    return out


# revision 9
# speedup vs baseline: 2.1792x; 2.1792x over previous
"""Grouped-index Conv1D (moe_routing) on 8 TRN2 NeuronCores.

Math:  out[b,d,t] = sum_c sum_k x[b,c,t+k] * W[gi[b,c],d,k] + count0[b]*bias[d]

Device algorithm (per core, 2 batches, data-parallel over batch):
  1. one-hot M[c,g] = (gi[b,c]==g) built on-chip (iota + tensor_scalar is_equal)
  2. S[g,t] = sum_c M[c,g]*x[c,t]          (PE: one-hot matmul, contraction=256)
  3. out[d,t] = sum_k Wk[g,d]^T S[g,t+k]   (PE: 7 shifted matmuls accumulated
                                            in PSUM, contraction=16)
  4. bias: counts via ones-matmul, count0*bias broadcast via 1-row matmul,
     fused add on DVE during PSUM->SBUF evacuation.
"""

import sys
import numpy as np

sys.path.insert(0, "/opt/trn_rl_repo")

BS, CH, T = 16, 256, 2048
G, D, K = 16, 64, 7
T_OUT = T - K + 1  # 2042
N_CORES = 8
BPC = BS // N_CORES  # batches per core = 2

# matmul input dtype for the two heavy stages: "f32" (exact, 4 cyc/row) or
# "f32r" (fast fp32 PE mode, 1 cyc/row at free>=256)
MM_DTYPE = "f32r"

_COMPILED = {}


def _build(mm_dtype: str):
    from concourse import bacc, tile
    import concourse.mybir as mybir

    f32 = mybir.dt.float32
    f32r = mybir.dt.float32r
    eq = mybir.AluOpType.is_equal
    add = mybir.AluOpType.add
    # dtype for tensors feeding the two heavy matmul stages; f32r inputs
    # must be produced by engine ops (which round), not raw DMA.
    mmdt = f32r if mm_dtype == "f32r" else f32
    use_r = mm_dtype == "f32r"

    nc = bacc.Bacc("TRN2", target_bir_lowering=False, debug=False,
                   num_devices=N_CORES)
    x_ext = nc.dram_tensor("x", [BPC, CH, T], f32, kind="ExternalInput").ap()
    gi_ext = nc.dram_tensor("gi", [BPC, 2, 128, 1], f32, kind="ExternalInput").ap()
    wt_ext = nc.dram_tensor("wt", [G, K * D], f32, kind="ExternalInput").ap()
    b_ext = nc.dram_tensor("bias", [1, D], f32, kind="ExternalInput").ap()
    out_ext = nc.dram_tensor("out", [BPC, D, T_OUT], f32, kind="ExternalOutput").ap()

    NCHUNK = T // 512  # 4

    with tile.TileContext(nc) as tc:
        with (
            tc.tile_pool(name="const", bufs=1) as cpool,
            tc.tile_pool(name="work", bufs=2) as wpool,
            tc.tile_pool(name="ps_pool", bufs=2, space="PSUM") as ppool,
            tc.tile_pool(name="psmall", bufs=1, space="PSUM") as spool,
            tc.tile_pool(name="po_pool", bufs=4, space="PSUM") as opool,
        ):
            wt_sb = cpool.tile([G, K * D], f32, name="wt_sb")
            nc.sync.dma_start(wt_sb[:], wt_ext[:])
            if use_r:
                wt_r = cpool.tile([G, K * D], f32r, name="wt_r")
                nc.vector.tensor_copy(wt_r[:], wt_sb[:])
            else:
                wt_r = wt_sb
            brow = cpool.tile([1, D], f32, name="brow")
            nc.sync.dma_start(brow[:], b_ext[:])
            iota_f = cpool.tile([128, G], f32, name="iota_f")
            nc.gpsimd.iota(iota_f[:], pattern=[[1, G]], base=0,
                           channel_multiplier=0,
                           allow_small_or_imprecise_dtypes=True)
            ones_col = cpool.tile([128, 1], f32, name="ones_col")
            nc.vector.memset(ones_col[:], 1.0)

            for b in range(BPC):
                # --- one-hot + counts + per-batch bias vector ---
                ms, ms_f = [], []
                for h in range(2):
                    gi_t = wpool.tile([128, 1], f32, name=f"gi{b}{h}",
                                      tag="gi", bufs=4)
                    nc.sync.dma_start(gi_t[:], gi_ext[b, h])
                    mf_t = wpool.tile([128, G], f32, name=f"mf{b}{h}",
                                      tag="mf", bufs=4)
                    nc.vector.tensor_scalar(out=mf_t[:], in0=iota_f[:],
                                            scalar1=gi_t[:, 0:1], scalar2=None,
                                            op0=eq)
                    ms_f.append(mf_t)
                    if use_r:
                        m_t = wpool.tile([128, G], f32r, name=f"m{b}{h}",
                                         tag="m", bufs=4)
                        nc.vector.tensor_copy(m_t[:], mf_t[:])
                    else:
                        m_t = mf_t
                    ms.append(m_t)

                pcnt = spool.tile([G, 1], f32, name=f"pcnt{b}", tag="pcnt")
                nc.tensor.matmul(pcnt[:], ms_f[0][:], ones_col[:],
                                 start=True, stop=False)
                nc.tensor.matmul(pcnt[:], ms_f[1][:], ones_col[:],
                                 start=False, stop=True)
                cnt_sb = wpool.tile([G, 1], f32, name=f"cnt{b}", tag="cnt")
                nc.vector.tensor_copy(cnt_sb[:], pcnt[:])
                pbc = spool.tile([D, 1], f32, name=f"pbc{b}", tag="pbc")
                nc.tensor.matmul(pbc[:], brow[:], cnt_sb[0:1, 0:1],
                                 start=True, stop=True)
                bcnt = wpool.tile([D, 1], f32, name=f"bcnt{b}", tag="bcnt")
                nc.vector.tensor_copy(bcnt[:], pbc[:])

                # --- x tiles: 2 channel-halves x 2 column-pieces of 1024 ---
                xp = [[None, None], [None, None]]
                for h in range(2):
                    for p in range(2):
                        t_ = wpool.tile([128, 1024], f32, name=f"xp{b}{h}{p}",
                                        tag="xp", bufs=5)
                        nc.sync.dma_start(
                            t_[:],
                            x_ext[b, 128 * h:128 * (h + 1),
                                  1024 * p:1024 * (p + 1)])
                        if use_r:
                            # round to f32r on the (otherwise idle) scalar
                            # engine so the PE can run fast-FP32 matmuls
                            xr = wpool.tile([128, 1024], f32r,
                                            name=f"xr{b}{h}{p}", tag="xr",
                                            bufs=5)
                            nc.scalar.activation(
                                xr[:], t_[:],
                                mybir.ActivationFunctionType.Copy)
                            t_ = xr
                        xp[h][p] = t_

                # --- S = M^T @ X ---
                s_sb = wpool.tile([G, T], mmdt, name=f"s{b}", tag="s")
                for c in range(NCHUNK):
                    ps = ppool.tile([G, 512], f32, name=f"ps{b}{c}", tag="ps")
                    off = 512 * c
                    p, o = off // 1024, off % 1024
                    nc.tensor.matmul(ps[:], ms[0][:],
                                     xp[0][p][:, o:o + 512],
                                     start=True, stop=False)
                    nc.tensor.matmul(ps[:], ms[1][:],
                                     xp[1][p][:, o:o + 512],
                                     start=False, stop=True)
                    nc.vector.tensor_copy(s_sb[:, off:off + 512], ps[:])

                # --- conv: k-accumulated matmuls, then bias-add + store ---
                for c in range(NCHUNK):
                    c0 = 512 * c
                    L = min(512, T_OUT - c0)
                    po = opool.tile([D, 512], f32, name=f"po{b}{c}", tag="po")
                    for k in range(K):
                        nc.tensor.matmul(po[:, :L],
                                         wt_r[:, D * k:D * (k + 1)],
                                         s_sb[:, c0 + k:c0 + k + L],
                                         start=(k == 0), stop=(k == K - 1))
                    osb = wpool.tile([D, 512], f32, name=f"osb{b}{c}",
                                     tag="osb", bufs=3)
                    nc.vector.tensor_scalar(out=osb[:, :L], in0=po[:, :L],
                                            scalar1=bcnt[:, 0:1], scalar2=None,
                                            op0=add)
                    nc.sync.dma_start(out_ext[b, :, c0:c0 + L], osb[:, :L])

    nc.compile()
    return nc


def _get_nc(mm_dtype: str):
    if mm_dtype not in _COMPILED:
        _COMPILED[mm_dtype] = _build(mm_dtype)
    return _COMPILED[mm_dtype]


def _run(x, group_idxs, W, bias, mm_dtype=None, trace=False, tmpdir=None):
    from concourse.bass_utils import run_bass_kernel_spmd

    x = np.ascontiguousarray(np.asarray(x, dtype=np.float32))
    gi = np.asarray(group_idxs).astype(np.float32).reshape(BS, 2, 128, 1)
    W = np.asarray(W, dtype=np.float32)
    bias = np.asarray(bias, dtype=np.float32)
    # wt[g, k*64+d] = W[g,d,k]
    wt = np.ascontiguousarray(W.transpose(0, 2, 1).reshape(G, K * D))
    brow = np.ascontiguousarray(bias.reshape(1, D))

    nc = _get_nc(mm_dtype or MM_DTYPE)
    in_maps = []
    for i in range(N_CORES):
        sl = slice(i * BPC, (i + 1) * BPC)
        in_maps.append({
            "x": np.ascontiguousarray(x[sl]),
            "gi": np.ascontiguousarray(gi[sl]),
            "wt": wt,
            "bias": brow,
        })
    res = run_bass_kernel_spmd(nc, in_maps, core_ids=list(range(N_CORES)),
                               trace=trace, tmpdir=tmpdir)
    out = np.concatenate([r["out"] for r in res.results], axis=0)
    assert out.shape == (BS, D, T_OUT)
    return out.astype(np.float32), res


def kernel(x, group_idxs, W, bias):
    out, _ = _run(x, group_idxs, W, bias)
    return out
